# revision 42
# baseline (speedup 1.0000x reference)
"""Gaussian falloff vortex-velocity kernel for Trainium2 (8 NeuronCores).

Math: out[b,h,w,:] = sum_n tau_n * exp(-r2/sig_n^2) / sqrt(r2) * (d2, -d1)
with d1 = py - y_n, d2 = px - x_n, r2 = d1^2 + d2^2.

Device-side structure (per core, H split 8 ways):
  1. r2 via TensorE:  r2 = pp - 2y*py - 2x*px + (y^2+x^2+eps), expanded as an
     8-row fp16 matmul (hi/lo split of each operand keeps fp32-level accuracy;
     products are exact in fp16xfp16->fp32, accumulation is fp32 PSUM).
     Output tile: [128 particles, 1024 points] across 2 PSUM banks.
  2. ACT:  lt = Ln(r2_mm + vv)         (bias = vv = y^2+x^2+eps, per-partition)
     DVE:  wt = chalf * lt + r2_mm     (chalf = 0.5*sig^2)
     ACT:  g  = Exp(nisg * wt + nisg*vv) -> fp16   (nisg = -1/sig^2)
     which equals exp(-r2/sig^2)/sqrt(r2).
  3. S-sums via TensorE: [6,512] = [tau,tau*x,tau*y] (hi/lo fp16) contracted
     over 128 particles, accumulated over the 4 particle blocks in PSUM,
     partition-stacked at offsets {0,32} for the two point-tiles of a chunk.
  4. u = px*S0 - S1, v = S2 - py*S0 on DVE.
"""

import sys

import numpy as np

B, H, W, N = 2, 256, 256, 512
NCORES = 8
HPC = H // NCORES          # 32 rows per core
PPB = HPC * W              # 8192 points per batch per core
NPT = PPB // 512           # 16 point-tiles of 512 per batch
NK = N // 128              # 4 particle blocks
EPS = 4e-6                 # keeps matmul-expanded r2 strictly positive

_cache = {}


def _bass_modules():
    if "/opt/trn_rl_repo" not in sys.path:
        sys.path.insert(0, "/opt/trn_rl_repo")
    import concourse.bass as bass
    import concourse.mybir as mybir
    import concourse.tile as tile
    from concourse import bacc
    from concourse.bass_utils import run_bass_kernel_spmd

    return bass, mybir, tile, run_bass_kernel_spmd, bacc


def _patch_act_tables(mybir, bacc):
    """Make Ln and Exp resolve to the combined natural_log_exp_and_others
    table set. The default first-match selection alternates between the
    natural_log and exp_and_others sets, inserting a ~1.3us ACT_TABLE_LOAD
    before nearly every activation (162us of pure table reloads per run)."""
    import concourse.hw_specs as hw_specs

    if getattr(bacc, "_act_tables_patched", False):
        return
    AF = mybir.ActivationFunctionType
    orig = hw_specs.get_activation_tables

    def patched(module_arch):
        tabs = orig(module_arch)
        out = {}
        for name, fns in tabs.items():
            if name != "natural_log_exp_and_others" and (
                AF.Ln in fns or AF.Exp in fns
            ):
                fns = fns - {AF.Ln, AF.Exp}
            out[name] = fns
        return out

    bacc.get_activation_tables = patched
    bacc._act_tables_patched = True


def _build_nc():
    bass, mybir, tile, _, bacc = _bass_modules()
    _patch_act_tables(mybir, bacc)
    f32 = mybir.dt.float32
    f16 = mybir.dt.float16
    AF = mybir.ActivationFunctionType
    ALU = mybir.AluOpType

    nc = bacc.Bacc(None)
    # params cols 0:8 = 2/sig^2 per (b,k), 8: = [px | -py] point-major per b
    params_d = nc.declare_dram_parameter(
        "params", [128, 8 + B * 128], f32, isOutput=False
    )
    # w16: rows 0:10, cols 0:1024 = r2-matmul weights; then S weights
    w16_d = nc.declare_dram_parameter(
        "w16", [128, B * NK * 128 + B * NK * 8], f16, isOutput=False
    )
    xrows_d = nc.declare_dram_parameter("xrows", [B, 10, PPB], f16, isOutput=False)
    out_d = nc.declare_dram_parameter("out", [B, 2, 128, 64], f32, isOutput=True)

    with tile.TileContext(nc) as tc:
        with (
            tc.tile_pool(name="const", bufs=1) as cpool,
            tc.tile_pool(name="xrows", bufs=2) as xpool,
            tc.tile_pool(name="lt", bufs=3) as ltp,
            tc.tile_pool(name="wt", bufs=3) as wtp,
            tc.tile_pool(name="g", bufs=3) as gp,
            tc.tile_pool(name="stage", bufs=2) as stp,
            tc.tile_pool(name="sd", bufs=2) as sdp,
            tc.tile_pool(name="fin", bufs=2) as finp,
            tc.tile_pool(name="dscratch", bufs=2, space="DRAM") as dpool,
            tc.tile_pool(name="psr2", bufs=2, space=bass.MemorySpace.PSUM) as psr2,
            tc.tile_pool(name="psac", bufs=2, space=bass.MemorySpace.PSUM) as psac,
        ):
            WG0 = B * NK * 128
            w16 = cpool.tile([128, WG0 + B * NK * 8], f16)
            # r2-matmul weights are on the critical path to the first matmul
            nc.sync.dma_start(w16[0:10, 0:WG0], w16_d[0:10, 0:WG0])
            pall = cpool.tile([128, 8 + B * 128], f32)
            params = pall[:, 0:8]

            # chunks of point-tiles; 3 tiles = [128,1536] ACT ops, PSUM fits
            # psr2 (3 banks x 2 bufs) + psac (1 bank x 2 bufs) = 8 banks
            chunks = [
                (b, T0, ntil)
                for b in range(B)
                for T0, ntil in [(0, 3), (3, 3), (6, 3), (9, 3), (12, 3), (15, 1)]
            ]
            xbs, scs, sds = {}, {}, {}

            def stage_a(ch):
                b, T0, ntil = ch
                fd = 512 * ntil
                if T0 == 0:
                    xb = xpool.tile([10, PPB], f16, tag="xb", name=f"xb{b}")
                    nc.sync.dma_start(xb[:, 0:1536], xrows_d[b, :, 0:1536])
                    if b == 0:
                        nc.sync.dma_start(pall[:], params_d[:])
                    nc.sync.dma_start(xb[:, 1536:PPB], xrows_d[b, :, 1536:PPB])
                    if b == 0:
                        nc.sync.dma_start(w16[:, WG0:], w16_d[:, WG0:])
                    xbs[b] = xb
                    scs[b] = dpool.tile([8, PPB], f32, tag="sc", name=f"sc{b}")
                    sds[b] = sdp.tile(
                        [128, 8 * 64], f32, tag="sd", name=f"sd{b}"
                    )
                xb = xbs[b]
                sacc = psac.tile([128, 512], f32, tag="sacc", name=f"sacc{b}_{T0}")
                # wt' = (2/sig^2)*r2 + ln(r2), g = exp(-0.5*wt')
                wtb = wtp.tile(
                    [128, NK * 1536], f16, tag="wt", name=f"wtb{b}_{T0}"
                )
                for k in range(NK):
                    c = b * NK + k
                    r2t = psr2.tile(
                        [128, 1536], f32, tag="r2", name=f"r2t{b}_{T0}_{k}"
                    )
                    for t in range(ntil):
                        T = T0 + t
                        nc.tensor.matmul(
                            r2t[:, 512 * t : 512 * (t + 1)],
                            w16[0:10, 128 * c : 128 * (c + 1)],
                            xb[:, 512 * T : 512 * (T + 1)],
                            start=True,
                            stop=True,
                        )
                    lt = ltp.tile([128, 1536], f32, tag="lt", name="lt")
                    nc.scalar.activation(lt[:, 0:fd], r2t[:, 0:fd], AF.Ln)
                    nc.vector.scalar_tensor_tensor(
                        wtb[:, 1536 * k : 1536 * k + fd],
                        r2t[:, 0:fd], params[:, c : c + 1], lt[:, 0:fd],
                        ALU.mult, ALU.add,
                    )
                return sacc, wtb

            def stage_b(ch, sacc, wtb):
                b, T0, ntil = ch
                fd = 512 * ntil
                g = gp.tile([128, NK * 1536], f16, tag="g", name=f"g{b}_{T0}")
                if fd == 1536:
                    nc.scalar.activation(g[:], wtb[:], AF.Exp, scale=-0.5)
                else:
                    for k in range(NK):
                        nc.scalar.activation(
                            g[:, 1536 * k : 1536 * k + fd],
                            wtb[:, 1536 * k : 1536 * k + fd],
                            AF.Exp, scale=-0.5,
                        )
                for k in range(NK):
                    c = b * NK + k
                    for t in range(ntil):
                        nc.tensor.matmul(
                            sacc[32 * t : 32 * t + 8, :],
                            w16[:, WG0 + 8 * c : WG0 + 8 * (c + 1)],
                            g[:, 1536 * k + 512 * t : 1536 * k + 512 * (t + 1)],
                            start=(k == 0),
                            stop=(k == NK - 1),
                        )
                stage = stp.tile([128, 512], f32, tag="stage", name="stage")
                nc.vector.tensor_copy(
                    stage[0 : 32 * (ntil - 1) + 8, :],
                    sacc[0 : 32 * (ntil - 1) + 8, :],
                )
                # straight copy to DRAM bounce rows, per tile
                for t in range(ntil):
                    T = T0 + t
                    nc.sync.dma_start(
                        scs[b][:, 512 * T : 512 * (T + 1)],
                        stage[32 * t : 32 * t + 8, :],
                    )
                # per-chunk gather: sd_all[p, 64r+c] = sc[r, 64p+c]
                with tc.high_priority():
                    nc.sync.dma_start(
                        sds[b][8 * T0 : 8 * (T0 + ntil), :].rearrange(
                            "p (r c) -> p r c", c=64
                        ),
                        scs[b][:, 512 * T0 : 512 * (T0 + ntil)].rearrange(
                            "r (p c) -> p r c", c=64
                        ),
                    )
                if T0 + ntil == NPT:
                    finale(b)

            def finale(b):
                # Ssum = [S0|S0|S1'|S2], uv = [px|-py]*[S0|S0] + [S1'|S2]
                sd_all = sds[b]
                with tc.high_priority():
                    pf2 = pall[:, 8 + b * 128 : 8 + (b + 1) * 128]
                    ssum = finp.tile([128, 256], f32, tag="ssum", name="ssum")
                    nc.vector.tensor_add(
                        ssum[:], sd_all[:, 0:256], sd_all[:, 256:512]
                    )
                    m = finp.tile([128, 128], f32, tag="m", name="m")
                    nc.vector.tensor_mul(m[:], pf2, ssum[:, 0:128])
                    uv = finp.tile([128, 128], f32, tag="uv", name="uv")
                    nc.vector.tensor_add(uv[:], m[:], ssum[:, 128:256])
                    nc.gpsimd.dma_start(
                        out_d[b].rearrange("a p c -> p a c"),
                        uv[:].rearrange("p (a c) -> p a c", c=64),
                    )

            # software-pipeline: stage A of chunk i+1 before stage B of chunk i
            prev = None
            for ch in chunks:
                st = stage_a(ch)
                if prev is not None:
                    stage_b(*prev)
                prev = (ch, *st)
            stage_b(*prev)
    nc.compile()
    return nc


def _hl(a):
    """fp16 hi/lo split of an fp32 array."""
    h = a.astype(np.float16)
    l = (a - h.astype(np.float32)).astype(np.float16)
    return h, l


def _prep_inputs(vortex_feature, points):
    vf = np.asarray(vortex_feature, dtype=np.float32)
    pts_full = np.asarray(points, dtype=np.float32)

    y = vf[:, :, 0]
    x = vf[:, :, 1]
    tau = vf[:, :, 2]
    sig = vf[:, :, 3]
    sig2 = sig * sig
    vv = y * y + x * x + EPS
    nisg = -1.0 / sig2
    chalf = 0.5 * sig2

    def blk(a):  # [B, N] -> [128, B*NK] with col = b*NK+k
        return np.ascontiguousarray(
            a.reshape(B, NK, 128).transpose(2, 0, 1).reshape(128, B * NK)
        )

    params = np.zeros((128, 8 + B * 128), dtype=np.float32)
    params[:, 0:8] = blk(2.0 / sig2)

    wyh, wyl = _hl(-2.0 * y)
    wxh, wxl = _hl(-2.0 * x)
    vvh, vvl = _hl(vv)
    ones = np.ones_like(wyh)
    # row r of wr2 pairs with row r of xrows; big terms first so PSUM partial
    # sums cancel early (less fp32 accumulation error on near pairs)
    wstack = np.stack(
        [ones, wyh, wxh, vvh, ones, wyh, wyl, wxh, wxl, vvl], axis=0
    )  # [10, B, N]
    wr2 = np.ascontiguousarray(
        wstack.reshape(10, B, NK, 128).reshape(10, B * NK * 128).astype(np.float16)
    )

    # S columns: [S0, S0, S1', S2] with S1' = sum(-tau*x*g), S2 = sum(tau*y*g)
    # so uv = [px|-py] * [S0|S0] + [S1'|S2] gives (u, v) directly
    w0, w1, w2 = tau, -tau * x, tau * y
    w0h, w0l = _hl(w0)
    w1h, w1l = _hl(w1)
    w2h, w2l = _hl(w2)
    wgf = np.stack([w0h, w0h, w1h, w2h, w0l, w0l, w1l, w2l], axis=-1)  # [B,N,8]
    wg = np.ascontiguousarray(
        wgf.reshape(B, NK, 128, 8).transpose(2, 0, 1, 3).reshape(128, B * NK * 8)
    ).astype(np.float16)
    w16 = np.zeros((128, B * NK * 128 + B * NK * 8), dtype=np.float16)
    w16[0:10, 0 : B * NK * 128] = wr2
    w16[:, B * NK * 128 :] = wg

    in_maps = []
    for i in range(NCORES):
        sl = pts_full[:, i * HPC : (i + 1) * HPC]          # [B, 32, 256, 2]
        flat = sl.reshape(B, PPB, 2)
        py = flat[:, :, 0]
        px = flat[:, :, 1]
        ph, pl = _hl(py)
        qh, ql = _hl(px)
        pp = py * py + px * px
        pph, ppl = _hl(pp)
        one_r = np.ones_like(ph)
        xrows = np.ascontiguousarray(
            np.stack(
                [pph, ph, qh, one_r, ppl, pl, ph, ql, qh, one_r], axis=1
            )  # [B, 10, PPB]
        ).astype(np.float16)
        pts = flat.transpose(0, 2, 1).reshape(B, 2, 128, PPB // 128)
        pcore = params.copy()
        for b in range(B):
            pcore[:, 8 + b * 128 : 8 + b * 128 + 64] = pts[b, 1]   # px
            pcore[:, 8 + b * 128 + 64 : 8 + (b + 1) * 128] = -pts[b, 0]  # -py
        in_maps.append({"params": pcore, "w16": w16, "xrows": xrows})
    return in_maps


def _assemble(results):
    out = np.zeros((B, H, W, 2), dtype=np.float32)
    for i in range(NCORES):
        o = np.asarray(results[i]["out"])  # [B, 2, 128, 64]
        o = o.reshape(B, 2, PPB).transpose(0, 2, 1).reshape(B, HPC, W, 2)
        out[:, i * HPC : (i + 1) * HPC] = o
    return out


def _run(vortex_feature, points, trace=False):
    _, _, _, run_bass_kernel_spmd, _b = _bass_modules()
    if "nc" not in _cache:
        _cache["nc"] = _build_nc()
    in_maps = _prep_inputs(vortex_feature, points)
    res = run_bass_kernel_spmd(
        _cache["nc"], in_maps, list(range(NCORES)), trace=trace
    )
    return _assemble(res.results), res


def kernel(vortex_feature, points):
    out, _ = _run(vortex_feature, points, trace=False)
    return out


# revision 43
# speedup vs baseline: 1.1342x; 1.1342x over previous
"""Gaussian falloff vortex-velocity kernel for Trainium2 (8 NeuronCores).

Math: out[b,h,w,:] = sum_n tau_n * exp(-r2/sig_n^2) / sqrt(r2) * (d2, -d1)
with d1 = py - y_n, d2 = px - x_n, r2 = d1^2 + d2^2.

Device-side structure (per core, H split 8 ways):
  1. r2 via TensorE:  r2 = pp - 2y*py - 2x*px + (y^2+x^2+eps), expanded as an
     8-row fp16 matmul (hi/lo split of each operand keeps fp32-level accuracy;
     products are exact in fp16xfp16->fp32, accumulation is fp32 PSUM).
     Output tile: [128 particles, 1024 points] across 2 PSUM banks.
  2. ACT:  lt = Ln(r2_mm + vv)         (bias = vv = y^2+x^2+eps, per-partition)
     DVE:  wt = chalf * lt + r2_mm     (chalf = 0.5*sig^2)
     ACT:  g  = Exp(nisg * wt + nisg*vv) -> fp16   (nisg = -1/sig^2)
     which equals exp(-r2/sig^2)/sqrt(r2).
  3. S-sums via TensorE: [6,512] = [tau,tau*x,tau*y] (hi/lo fp16) contracted
     over 128 particles, accumulated over the 4 particle blocks in PSUM,
     partition-stacked at offsets {0,32} for the two point-tiles of a chunk.
  4. u = px*S0 - S1, v = S2 - py*S0 on DVE.
"""

import sys

import numpy as np

B, H, W, N = 2, 256, 256, 512
NCORES = 8
HPC = H // NCORES          # 32 rows per core
PPB = HPC * W              # 8192 points per batch per core
NPT = PPB // 512           # 16 point-tiles of 512 per batch
NK = N // 128              # 4 particle blocks
EPS = 4e-6                 # keeps matmul-expanded r2 strictly positive

_cache = {}


def _bass_modules():
    if "/opt/trn_rl_repo" not in sys.path:
        sys.path.insert(0, "/opt/trn_rl_repo")
    import concourse.bass as bass
    import concourse.mybir as mybir
    import concourse.tile as tile
    from concourse import bacc
    from concourse.bass_utils import run_bass_kernel_spmd

    return bass, mybir, tile, run_bass_kernel_spmd, bacc


def _patch_act_tables(mybir, bacc):
    """Make Ln and Exp resolve to the combined natural_log_exp_and_others
    table set. The default first-match selection alternates between the
    natural_log and exp_and_others sets, inserting a ~1.3us ACT_TABLE_LOAD
    before nearly every activation (162us of pure table reloads per run)."""
    import concourse.hw_specs as hw_specs

    if getattr(bacc, "_act_tables_patched", False):
        return
    AF = mybir.ActivationFunctionType
    orig = hw_specs.get_activation_tables

    def patched(module_arch):
        tabs = orig(module_arch)
        out = {}
        for name, fns in tabs.items():
            if name != "natural_log_exp_and_others" and (
                AF.Ln in fns or AF.Exp in fns
            ):
                fns = fns - {AF.Ln, AF.Exp}
            out[name] = fns
        return out

    bacc.get_activation_tables = patched
    bacc._act_tables_patched = True


def _build_nc():
    bass, mybir, tile, _, bacc = _bass_modules()
    _patch_act_tables(mybir, bacc)
    f32 = mybir.dt.float32
    f16 = mybir.dt.float16
    AF = mybir.ActivationFunctionType
    ALU = mybir.AluOpType

    nc = bacc.Bacc(None)
    # params cols 0:8 = 2/sig^2 per (b,k), 8: = [px | -py] point-major per b
    params_d = nc.declare_dram_parameter(
        "params", [128, 8 + B * 128], f32, isOutput=False
    )
    # w16: rows 0:10, cols 0:1024 = r2-matmul weights; then S weights
    w16_d = nc.declare_dram_parameter(
        "w16", [128, B * NK * 128 + B * NK * 8], f16, isOutput=False
    )
    xrows_d = nc.declare_dram_parameter("xrows", [B, 10, PPB], f16, isOutput=False)
    out_d = nc.declare_dram_parameter("out", [B, 2, 128, 64], f32, isOutput=True)

    with tile.TileContext(nc) as tc:
        with (
            tc.tile_pool(name="const", bufs=1) as cpool,
            tc.tile_pool(name="xrows", bufs=2) as xpool,
            tc.tile_pool(name="lt", bufs=3) as ltp,
            tc.tile_pool(name="wt", bufs=3) as wtp,
            tc.tile_pool(name="g", bufs=3) as gp,
            tc.tile_pool(name="stage", bufs=2) as stp,
            tc.tile_pool(name="sd", bufs=2) as sdp,
            tc.tile_pool(name="fin", bufs=2) as finp,
            tc.tile_pool(name="dscratch", bufs=2, space="DRAM") as dpool,
            tc.tile_pool(name="psr2", bufs=3, space=bass.MemorySpace.PSUM) as psr2,
            tc.tile_pool(name="psac", bufs=2, space=bass.MemorySpace.PSUM) as psac,
        ):
            WG0 = B * NK * 128
            w16 = cpool.tile([128, WG0 + B * NK * 8], f16)
            # r2-matmul weights are on the critical path to the first matmul
            nc.sync.dma_start(w16[0:10, 0:WG0], w16_d[0:10, 0:WG0])
            pall = cpool.tile([128, 8 + B * 128], f32)
            params = pall[:, 0:8]

            # chunks of point-tiles; 3 tiles = [128,1536] ACT ops, PSUM fits
            # psr2 (3 banks x 2 bufs) + psac (1 bank x 2 bufs) = 8 banks
            chunks = [(b, 2 * i, 2) for b in range(B) for i in range(NPT // 2)]
            xbs, scs, sds = {}, {}, {}

            def stage_a(ch):
                b, T0, ntil = ch
                fd = 512 * ntil
                if T0 == 0:
                    xb = xpool.tile([10, PPB], f16, tag="xb", name=f"xb{b}")
                    nc.sync.dma_start(xb[:, 0:1536], xrows_d[b, :, 0:1536])
                    if b == 0:
                        nc.sync.dma_start(pall[:], params_d[:])
                    nc.sync.dma_start(xb[:, 1536:PPB], xrows_d[b, :, 1536:PPB])
                    if b == 0:
                        nc.sync.dma_start(w16[:, WG0:], w16_d[:, WG0:])
                    xbs[b] = xb
                    scs[b] = dpool.tile([8, PPB], f32, tag="sc", name=f"sc{b}")
                    sds[b] = sdp.tile(
                        [128, 8 * 64], f32, tag="sd", name=f"sd{b}"
                    )
                xb = xbs[b]
                sacc = psac.tile([128, 512], f32, tag="sacc", name=f"sacc{b}_{T0}")
                # wt' = (2/sig^2)*r2 + ln(r2), g = exp(-0.5*wt')
                wtb = wtp.tile(
                    [128, NK * 1024], f16, tag="wt", name=f"wtb{b}_{T0}"
                )
                for k in range(NK):
                    c = b * NK + k
                    r2t = psr2.tile(
                        [128, 1024], f32, tag="r2", name=f"r2t{b}_{T0}_{k}"
                    )
                    for t in range(ntil):
                        T = T0 + t
                        nc.tensor.matmul(
                            r2t[:, 512 * t : 512 * (t + 1)],
                            w16[0:10, 128 * c : 128 * (c + 1)],
                            xb[:, 512 * T : 512 * (T + 1)],
                            start=True,
                            stop=True,
                        )
                    lt = ltp.tile([128, 1024], f32, tag="lt", name="lt")
                    nc.scalar.activation(lt[:, 0:fd], r2t[:, 0:fd], AF.Ln)
                    nc.vector.scalar_tensor_tensor(
                        wtb[:, 1024 * k : 1024 * k + fd],
                        r2t[:, 0:fd], params[:, c : c + 1], lt[:, 0:fd],
                        ALU.mult, ALU.add,
                    )
                return sacc, wtb

            def stage_b(ch, sacc, wtb):
                b, T0, ntil = ch
                fd = 512 * ntil
                g = gp.tile([128, NK * 1024], f16, tag="g", name=f"g{b}_{T0}")
                nc.scalar.activation(g[:], wtb[:], AF.Exp, scale=-0.5)
                for k in range(NK):
                    c = b * NK + k
                    for t in range(ntil):
                        nc.tensor.matmul(
                            sacc[32 * t : 32 * t + 8, :],
                            w16[:, WG0 + 8 * c : WG0 + 8 * (c + 1)],
                            g[:, 1024 * k + 512 * t : 1024 * k + 512 * (t + 1)],
                            start=(k == 0),
                            stop=(k == NK - 1),
                        )
                stage = stp.tile([128, 512], f32, tag="stage", name="stage")
                nc.vector.tensor_copy(
                    stage[0 : 32 * (ntil - 1) + 8, :],
                    sacc[0 : 32 * (ntil - 1) + 8, :],
                )
                # straight copy to DRAM bounce rows, per tile
                for t in range(ntil):
                    T = T0 + t
                    nc.sync.dma_start(
                        scs[b][:, 512 * T : 512 * (T + 1)],
                        stage[32 * t : 32 * t + 8, :],
                    )
                # per-chunk gather: sd_all[p, 64r+c] = sc[r, 64p+c]
                with tc.high_priority():
                    nc.sync.dma_start(
                        sds[b][8 * T0 : 8 * (T0 + ntil), :].rearrange(
                            "p (r c) -> p r c", c=64
                        ),
                        scs[b][:, 512 * T0 : 512 * (T0 + ntil)].rearrange(
                            "r (p c) -> p r c", c=64
                        ),
                    )
                if T0 + ntil == NPT:
                    finale(b)

            def finale(b):
                # Ssum = [S0|S0|S1'|S2], uv = [px|-py]*[S0|S0] + [S1'|S2]
                sd_all = sds[b]
                with tc.high_priority():
                    pf2 = pall[:, 8 + b * 128 : 8 + (b + 1) * 128]
                    ssum = finp.tile([128, 256], f32, tag="ssum", name="ssum")
                    nc.vector.tensor_add(
                        ssum[:], sd_all[:, 0:256], sd_all[:, 256:512]
                    )
                    m = finp.tile([128, 128], f32, tag="m", name="m")
                    nc.vector.tensor_mul(m[:], pf2, ssum[:, 0:128])
                    uv = finp.tile([128, 128], f32, tag="uv", name="uv")
                    nc.vector.tensor_add(uv[:], m[:], ssum[:, 128:256])
                    nc.gpsimd.dma_start(
                        out_d[b].rearrange("a p c -> p a c"),
                        uv[:].rearrange("p (a c) -> p a c", c=64),
                    )

            # software-pipeline: stage A of chunk i+1 before stage B of chunk i
            prev = None
            for ch in chunks:
                st = stage_a(ch)
                if prev is not None:
                    stage_b(*prev)
                prev = (ch, *st)
            stage_b(*prev)
    nc.compile()
    return nc


def _hl(a):
    """fp16 hi/lo split of an fp32 array."""
    h = a.astype(np.float16)
    l = (a - h.astype(np.float32)).astype(np.float16)
    return h, l


def _prep_inputs(vortex_feature, points):
    vf = np.asarray(vortex_feature, dtype=np.float32)
    pts_full = np.asarray(points, dtype=np.float32)

    y = vf[:, :, 0]
    x = vf[:, :, 1]
    tau = vf[:, :, 2]
    sig = vf[:, :, 3]
    sig2 = sig * sig
    vv = y * y + x * x + EPS
    nisg = -1.0 / sig2
    chalf = 0.5 * sig2

    def blk(a):  # [B, N] -> [128, B*NK] with col = b*NK+k
        return np.ascontiguousarray(
            a.reshape(B, NK, 128).transpose(2, 0, 1).reshape(128, B * NK)
        )

    params = np.zeros((128, 8 + B * 128), dtype=np.float32)
    params[:, 0:8] = blk(2.0 / sig2)

    wyh, wyl = _hl(-2.0 * y)
    wxh, wxl = _hl(-2.0 * x)
    vvh, vvl = _hl(vv)
    ones = np.ones_like(wyh)
    # row r of wr2 pairs with row r of xrows; big terms first so PSUM partial
    # sums cancel early (less fp32 accumulation error on near pairs)
    wstack = np.stack(
        [ones, wyh, wxh, vvh, ones, wyh, wyl, wxh, wxl, vvl], axis=0
    )  # [10, B, N]
    wr2 = np.ascontiguousarray(
        wstack.reshape(10, B, NK, 128).reshape(10, B * NK * 128).astype(np.float16)
    )

    # S columns: [S0, S0, S1', S2] with S1' = sum(-tau*x*g), S2 = sum(tau*y*g)
    # so uv = [px|-py] * [S0|S0] + [S1'|S2] gives (u, v) directly
    w0, w1, w2 = tau, -tau * x, tau * y
    w0h, w0l = _hl(w0)
    w1h, w1l = _hl(w1)
    w2h, w2l = _hl(w2)
    wgf = np.stack([w0h, w0h, w1h, w2h, w0l, w0l, w1l, w2l], axis=-1)  # [B,N,8]
    wg = np.ascontiguousarray(
        wgf.reshape(B, NK, 128, 8).transpose(2, 0, 1, 3).reshape(128, B * NK * 8)
    ).astype(np.float16)
    w16 = np.zeros((128, B * NK * 128 + B * NK * 8), dtype=np.float16)
    w16[0:10, 0 : B * NK * 128] = wr2
    w16[:, B * NK * 128 :] = wg

    in_maps = []
    for i in range(NCORES):
        sl = pts_full[:, i * HPC : (i + 1) * HPC]          # [B, 32, 256, 2]
        flat = sl.reshape(B, PPB, 2)
        py = flat[:, :, 0]
        px = flat[:, :, 1]
        ph, pl = _hl(py)
        qh, ql = _hl(px)
        pp = py * py + px * px
        pph, ppl = _hl(pp)
        one_r = np.ones_like(ph)
        xrows = np.ascontiguousarray(
            np.stack(
                [pph, ph, qh, one_r, ppl, pl, ph, ql, qh, one_r], axis=1
            )  # [B, 10, PPB]
        ).astype(np.float16)
        pts = flat.transpose(0, 2, 1).reshape(B, 2, 128, PPB // 128)
        pcore = params.copy()
        for b in range(B):
            pcore[:, 8 + b * 128 : 8 + b * 128 + 64] = pts[b, 1]   # px
            pcore[:, 8 + b * 128 + 64 : 8 + (b + 1) * 128] = -pts[b, 0]  # -py
        in_maps.append({"params": pcore, "w16": w16, "xrows": xrows})
    return in_maps


def _assemble(results):
    out = np.zeros((B, H, W, 2), dtype=np.float32)
    for i in range(NCORES):
        o = np.asarray(results[i]["out"])  # [B, 2, 128, 64]
        o = o.reshape(B, 2, PPB).transpose(0, 2, 1).reshape(B, HPC, W, 2)
        out[:, i * HPC : (i + 1) * HPC] = o
    return out


def _run(vortex_feature, points, trace=False):
    _, _, _, run_bass_kernel_spmd, _b = _bass_modules()
    if "nc" not in _cache:
        _cache["nc"] = _build_nc()
    in_maps = _prep_inputs(vortex_feature, points)
    res = run_bass_kernel_spmd(
        _cache["nc"], in_maps, list(range(NCORES)), trace=trace
    )
    return _assemble(res.results), res


def kernel(vortex_feature, points):
    out, _ = _run(vortex_feature, points, trace=False)
    return out
